# revision 26
# baseline (speedup 1.0000x reference)
"""Trainium2 Bass kernel for ComplexDFT256 — bf16 radix-2 version.

Math: the 256-point complex DFT out = z @ M (z = [xr | xi], M the
512x512 real form of the DFT) is split radix-2 over time samples:
  X[k]      = E[k] + G[k]        k = 0..127
  X[k+128]  = E[k] - G[k]
with E = DFT-128 of even samples and G = (twiddle * DFT-128) of odd
samples; the twiddles fold into G's matrix on the host, so on-device
this is two [B,256]@[256,256] matmuls (half the PE work of the dense
form) plus one add + one sub per output tile (DVE).

Everything streams in bf16 (inputs pre-cast on host, outputs cast by
the DVE butterfly, PSUM accumulates fp32), halving HBM traffic vs
fp32r: 8 MB in + 8 MB out per core.  Measured end-to-end error vs the
fp32 reference ~2.7e-3 of output norm (tolerance 2e-2).

Sharding: pure data parallel over batch across 8 NeuronCores (8192
rows each).  Host pre-permutes columns to [even | odd] order and
transposes to [512, B] so the contraction dim lands on SBUF partitions
with contiguous DMA.

Output leaves the device in a [64, 128, 2, 512] layout (one DMA per
256-row group, partition-major) and is un-permuted on the host.
"""
import numpy as np
import ml_dtypes

import concourse.bacc as bacc
import concourse.mybir as mybir
import concourse.tile as tile
from concourse.bass_utils import run_bass_kernel_spmd

N_CORES = 8
BATCH = 65536
FFT = 256
C = 2 * FFT            # contraction dim = 512 ([even 256 | odd 256])
J = 2 * FFT            # output features = 512
B_SHARD = BATCH // N_CORES   # 8192
GROUP_B = 512          # batch rows per matmul group (moving free dim)
N_GROUPS = B_SHARD // GROUP_B             # 16
# ramped chunk sizes: small first chunks shrink the pipeline-fill time
# (first matmul starts after a 0.5 MB load instead of 2.1 MB)
CHUNKS = [512, 512, 1024, 2048, 2048, 2048]
assert sum(CHUNKS) == B_SHARD
CHUNK_MAX = max(CHUNKS)

BF16 = ml_dtypes.bfloat16

_cache = {}


def _build_nc(reps: int = 1, unroll: bool = False):
    nc = bacc.Bacc("TRN2", target_bir_lowering=False, debug=False,
                   num_devices=N_CORES)
    f32 = mybir.dt.float32
    bf16 = mybir.dt.bfloat16

    zt_dram = nc.dram_tensor("zt", [C, B_SHARD], bf16, kind="ExternalInput")
    m_dram = nc.dram_tensor("m", [C, 256], bf16, kind="ExternalInput")
    # transposed output: [group, j-partition, lo/hi, Re/Im, batch-in-group];
    # host un-permutes
    out_dram = nc.dram_tensor("out", [N_GROUPS, 128, 2, 2, GROUP_B], bf16,
                              kind="ExternalOutput")

    with tile.TileContext(nc) as tc:
        with (
            tc.tile_pool(name="mpool", bufs=1) as mpool,
            tc.tile_pool(name="zpool", bufs=3) as zpool,
            tc.tile_pool(name="gpool", bufs=4) as gpool,
            tc.tile_pool(name="opool", bufs=6) as opool,
            tc.tile_pool(name="psum", bufs=2, space="PSUM") as psum_pool,
        ):
            m_sb = []
            for k in range(4):
                mt = mpool.tile([128, 256], bf16, tag=f"m{k}")
                # SWDGE: keeps the m loads off the SP queue so the first
                # zt chunk streams in parallel
                nc.gpsimd.dma_start(mt[:], m_dram[k * 128:(k + 1) * 128, :])
                m_sb.append(mt)

            def body():
                G = 0
                off = 0
                for cb in CHUNKS:
                    zt_sb = zpool.tile([128, 4, CHUNK_MAX], bf16, tag="zt")
                    for k in range(4):
                        nc.sync.dma_start(
                            zt_sb[:, k, 0:cb],
                            zt_dram[k * 128:(k + 1) * 128, off:off + cb],
                        )
                    for g in range(cb // GROUP_B):
                        # Transposed matmuls: stationary = 128x128 M
                        # block, moving = 512 batch columns.  Y[:, 2q+jt]
                        # = [128 j, 512 b] with q=0 even-half (E), q=1
                        # odd-half (G), jt=0 Re / jt=1 Im columns.
                        Y = psum_pool.tile([128, 4, GROUP_B], f32,
                                           tag="acc")
                        csl = slice(g * GROUP_B, (g + 1) * GROUP_B)
                        for q in range(2):
                            for jt in range(2):
                                jsl = slice(jt * 128, (jt + 1) * 128)
                                nc.tensor.matmul(
                                    Y[:, 2 * q + jt, :],
                                    m_sb[2 * q][:, jsl],
                                    zt_sb[:, 2 * q, csl],
                                    start=True, stop=False)
                                nc.tensor.matmul(
                                    Y[:, 2 * q + jt, :],
                                    m_sb[2 * q + 1][:, jsl],
                                    zt_sb[:, 2 * q + 1, csl],
                                    start=False, stop=True)
                        # one contiguous PSUM->SBUF drain on ACT (bf16),
                        # then both DVE butterfly ops run all-SBUF in
                        # bf16 (DVE TensorTensor may read only one PSUM
                        # input; all-SBUF 16-bit ops get DVE 2x mode)
                        stg = gpool.tile([128, 4, GROUP_B], bf16,
                                         tag="stg")
                        nc.scalar.copy(stg[:], Y[:])
                        te = stg[:, 0:2, :]
                        tg = stg[:, 2:4, :]
                        # out_sb[:, a, jt, :]: a=0 -> X[k]=E+G, a=1 ->
                        # X[k+128]=E-G; contiguous 1024-elem DVE writes
                        out_sb = opool.tile([128, 2, 2, GROUP_B], bf16,
                                            tag="out")
                        nc.vector.tensor_add(out_sb[:, 0, :, :], te, tg)
                        nc.vector.tensor_sub(out_sb[:, 1, :, :], te, tg)
                        # stores on the SP HWDGE queue with the loads
                        nc.sync.dma_start(out_dram[G], out_sb[:])
                        G += 1
                    off += cb

            if reps == 1:
                body()
            elif unroll:
                for _ in range(reps):
                    body()
            else:
                with tc.For_i(0, reps, 1):
                    body()

    nc.compile()
    return nc


def _get_nc():
    if "nc" not in _cache:
        _cache["nc"] = _build_nc()
    return _cache["nc"]


def _prepare_in_maps(x, cos_kernel, sin_kernel):
    x = np.asarray(x, dtype=np.float32)
    cos = np.asarray(cos_kernel, dtype=np.float32)
    sin = np.asarray(sin_kernel, dtype=np.float32)

    m = np.empty((C, J), dtype=np.float32)
    m[:FFT, :FFT] = cos.T
    m[:FFT, FFT:] = sin.T
    m[FFT:, :FFT] = -sin.T
    m[FFT:, FFT:] = cos.T

    # radix-2: even/odd sample rows; cols k<128 of both Re and Im halves
    # (cols k+128 equal these up to the sign of the odd-row block)
    rows_e = np.concatenate([np.arange(0, 256, 2), np.arange(256, 512, 2)])
    rows_o = rows_e + 1
    cols_lo = np.concatenate([np.arange(0, 128), np.arange(256, 384)])
    me = m[np.ix_(rows_e, cols_lo)]     # [256, 256]
    mg = m[np.ix_(rows_o, cols_lo)]     # [256, 256]
    m_dev = np.concatenate([me, mg], axis=0).astype(BF16)  # [512, 256]

    z = x.reshape(BATCH, C)[:, np.concatenate([rows_e, rows_o])].astype(BF16)
    zt = np.ascontiguousarray(z.view(np.uint16).T)  # [512, BATCH] as u16

    in_maps = []
    for c in range(N_CORES):
        shard = np.ascontiguousarray(
            zt[:, c * B_SHARD:(c + 1) * B_SHARD]).view(BF16)
        in_maps.append({"zt": shard, "m": m_dev})
    return in_maps


def _run(in_maps, trace=False):
    nc = _get_nc()
    return run_bass_kernel_spmd(nc, in_maps, list(range(N_CORES)), trace=trace)


def kernel(x, cos_kernel, sin_kernel):
    in_maps = _prepare_in_maps(x, cos_kernel, sin_kernel)
    res = _run(in_maps)
    outs = []
    for r in res.results:
        # [G, p, a, q, b] bf16 (transposed): row = G*GROUP_B + b,
        # col = q*256 + a*128 + p
        o = np.asarray(r["out"]).view(np.uint16)
        o = o.transpose(0, 4, 3, 2, 1).reshape(B_SHARD, J)  # (G,b,q,a,p)
        outs.append(o)
    out = np.concatenate(outs, axis=0).view(BF16).astype(np.float32)
    return out.reshape(BATCH, J, 1)


# revision 29
# speedup vs baseline: 1.0715x; 1.0715x over previous
"""Trainium2 Bass kernel for ComplexDFT256 — bf16 radix-2 version.

Math: the 256-point complex DFT out = z @ M (z = [xr | xi], M the
512x512 real form of the DFT) is split radix-2 over time samples:
  X[k]      = E[k] + G[k]        k = 0..127
  X[k+128]  = E[k] - G[k]
with E = DFT-128 of even samples and G = (twiddle * DFT-128) of odd
samples; the twiddles fold into G's matrix on the host, so on-device
this is two [B,256]@[256,256] matmuls (half the PE work of the dense
form) plus one add + one sub per output tile (the DVE butterfly).

Everything streams in bf16 (inputs pre-cast on host, PSUM accumulates
fp32, ACT drains PSUM to bf16), halving HBM traffic vs fp32r: 8 MB in
+ 8 MB out per core — the DMA roofline (~332 GB/s effective) is the
bottleneck.  Measured end-to-end error vs the fp32 reference ~3.2e-3
of output norm (tolerance 2e-2).

Structure per 512-row group: 8 transposed matmuls (stationary = 128x128
M block, moving = 512 batch columns, so PE instruction count stays
low), one contiguous 4-bank PSUM->SBUF drain on ACT (DVE TensorTensor
may read only one PSUM input), two all-SBUF bf16 DVE butterfly ops
(2x perf mode), one store.  Loads issue from the SP HWDGE queue and
stores from the DVE queue — a store on the SP queue would head-of-line
block the next chunk's loads behind the store's compute-chain sem wait.
Ramped chunk sizes shrink the pipeline-fill before the first matmul.

Sharding: pure data parallel over batch across 8 NeuronCores (8192
rows each).  Host pre-permutes columns to [even | odd] order and
transposes to [512, B] so the contraction dim lands on SBUF partitions
with contiguous DMA; the transposed device output layout is un-permuted
on the host.
"""
import numpy as np
import ml_dtypes

import concourse.bacc as bacc
import concourse.mybir as mybir
import concourse.tile as tile
from concourse.bass_utils import run_bass_kernel_spmd

N_CORES = 8
BATCH = 65536
FFT = 256
C = 2 * FFT            # contraction dim = 512 ([even 256 | odd 256])
J = 2 * FFT            # output features = 512
B_SHARD = BATCH // N_CORES   # 8192
GROUP_B = 512          # batch rows per matmul group (moving free dim)
N_GROUPS = B_SHARD // GROUP_B             # 16
# ramped chunk sizes: small first chunks shrink the pipeline-fill time
# (first matmul starts after a 0.5 MB load instead of 2.1 MB)
CHUNKS = [512, 512, 1024, 2048, 2048, 2048]
assert sum(CHUNKS) == B_SHARD
CHUNK_MAX = max(CHUNKS)

BF16 = ml_dtypes.bfloat16

_cache = {}


def _build_nc(reps: int = 1, unroll: bool = False):
    nc = bacc.Bacc("TRN2", target_bir_lowering=False, debug=False,
                   num_devices=N_CORES)
    f32 = mybir.dt.float32
    bf16 = mybir.dt.bfloat16

    zt_dram = nc.dram_tensor("zt", [C, B_SHARD], bf16, kind="ExternalInput")
    m_dram = nc.dram_tensor("m", [C, 256], bf16, kind="ExternalInput")
    # transposed output: [group, j-partition, lo/hi, Re/Im, batch-in-group];
    # host un-permutes
    out_dram = nc.dram_tensor("out", [N_GROUPS, 128, 2, 2, GROUP_B], bf16,
                              kind="ExternalOutput")

    with tile.TileContext(nc) as tc:
        with (
            tc.tile_pool(name="mpool", bufs=1) as mpool,
            tc.tile_pool(name="zpool", bufs=3) as zpool,
            tc.tile_pool(name="gpool", bufs=4) as gpool,
            tc.tile_pool(name="opool", bufs=6) as opool,
            tc.tile_pool(name="psum", bufs=2, space="PSUM") as psum_pool,
        ):
            m_sb = []
            for k in range(4):
                mt = mpool.tile([128, 256], bf16, tag=f"m{k}")
                # SWDGE: keeps the m loads off the SP queue so the first
                # zt chunk streams in parallel
                nc.gpsimd.dma_start(mt[:], m_dram[k * 128:(k + 1) * 128, :])
                m_sb.append(mt)

            def body():
                G = 0
                off = 0
                for cb in CHUNKS:
                    zt_sb = zpool.tile([128, 4, CHUNK_MAX], bf16, tag="zt")
                    for k in range(4):
                        nc.sync.dma_start(
                            zt_sb[:, k, 0:cb],
                            zt_dram[k * 128:(k + 1) * 128, off:off + cb],
                        )
                    for g in range(cb // GROUP_B):
                        # Transposed matmuls: stationary = 128x128 M
                        # block, moving = 512 batch columns.  Y[:, 2q+jt]
                        # = [128 j, 512 b] with q=0 even-half (E), q=1
                        # odd-half (G), jt=0 Re / jt=1 Im columns.
                        Y = psum_pool.tile([128, 4, GROUP_B], f32,
                                           tag="acc")
                        csl = slice(g * GROUP_B, (g + 1) * GROUP_B)
                        for q in range(2):
                            for jt in range(2):
                                jsl = slice(jt * 128, (jt + 1) * 128)
                                nc.tensor.matmul(
                                    Y[:, 2 * q + jt, :],
                                    m_sb[2 * q][:, jsl],
                                    zt_sb[:, 2 * q, csl],
                                    start=True, stop=False)
                                nc.tensor.matmul(
                                    Y[:, 2 * q + jt, :],
                                    m_sb[2 * q + 1][:, jsl],
                                    zt_sb[:, 2 * q + 1, csl],
                                    start=False, stop=True)
                        # one contiguous PSUM->SBUF drain on ACT (bf16),
                        # then both DVE butterfly ops run all-SBUF in
                        # bf16 (DVE TensorTensor may read only one PSUM
                        # input; all-SBUF 16-bit ops get DVE 2x mode)
                        stg = gpool.tile([128, 4, GROUP_B], bf16,
                                         tag="stg")
                        nc.scalar.copy(stg[:], Y[:])
                        te = stg[:, 0:2, :]
                        tg = stg[:, 2:4, :]
                        # out_sb[:, a, jt, :]: a=0 -> X[k]=E+G, a=1 ->
                        # X[k+128]=E-G; contiguous 1024-elem DVE writes
                        out_sb = opool.tile([128, 2, 2, GROUP_B], bf16,
                                            tag="out")
                        nc.vector.tensor_add(out_sb[:, 0, :, :], te, tg)
                        nc.vector.tensor_sub(out_sb[:, 1, :, :], te, tg)
                        # stores issue from the ACT HWDGE queue so they
                        # never head-of-line-block the SP queue's loads
                        nc.scalar.dma_start(out_dram[G], out_sb[:])
                        G += 1
                    off += cb

            if reps == 1:
                body()
            elif unroll:
                for _ in range(reps):
                    body()
            else:
                with tc.For_i(0, reps, 1):
                    body()

    nc.compile()
    return nc


def _get_nc():
    if "nc" not in _cache:
        _cache["nc"] = _build_nc()
    return _cache["nc"]


def _prepare_in_maps(x, cos_kernel, sin_kernel):
    x = np.asarray(x, dtype=np.float32)
    cos = np.asarray(cos_kernel, dtype=np.float32)
    sin = np.asarray(sin_kernel, dtype=np.float32)

    m = np.empty((C, J), dtype=np.float32)
    m[:FFT, :FFT] = cos.T
    m[:FFT, FFT:] = sin.T
    m[FFT:, :FFT] = -sin.T
    m[FFT:, FFT:] = cos.T

    # radix-2: even/odd sample rows; cols k<128 of both Re and Im halves
    # (cols k+128 equal these up to the sign of the odd-row block)
    rows_e = np.concatenate([np.arange(0, 256, 2), np.arange(256, 512, 2)])
    rows_o = rows_e + 1
    cols_lo = np.concatenate([np.arange(0, 128), np.arange(256, 384)])
    me = m[np.ix_(rows_e, cols_lo)]     # [256, 256]
    mg = m[np.ix_(rows_o, cols_lo)]     # [256, 256]
    m_dev = np.concatenate([me, mg], axis=0).astype(BF16)  # [512, 256]

    z = x.reshape(BATCH, C)[:, np.concatenate([rows_e, rows_o])].astype(BF16)
    zt = np.ascontiguousarray(z.view(np.uint16).T)  # [512, BATCH] as u16

    in_maps = []
    for c in range(N_CORES):
        shard = np.ascontiguousarray(
            zt[:, c * B_SHARD:(c + 1) * B_SHARD]).view(BF16)
        in_maps.append({"zt": shard, "m": m_dev})
    return in_maps


def _run(in_maps, trace=False):
    nc = _get_nc()
    return run_bass_kernel_spmd(nc, in_maps, list(range(N_CORES)), trace=trace)


def kernel(x, cos_kernel, sin_kernel):
    in_maps = _prepare_in_maps(x, cos_kernel, sin_kernel)
    res = _run(in_maps)
    outs = []
    for r in res.results:
        # [G, p, a, q, b] bf16 (transposed): row = G*GROUP_B + b,
        # col = q*256 + a*128 + p
        o = np.asarray(r["out"]).view(np.uint16)
        o = o.transpose(0, 4, 3, 2, 1).reshape(B_SHARD, J)  # (G,b,q,a,p)
        outs.append(o)
    out = np.concatenate(outs, axis=0).view(BF16).astype(np.float32)
    return out.reshape(BATCH, J, 1)
